# revision 12
# baseline (speedup 1.0000x reference)
"""Trainium2 Bass kernel for nn_BinaryProductCodeMemoryLayer.

Product-key-memory layer: low-rank query projection -> per-bucket 2-way key
scores -> exact top-32 smallest subset-sum beam search over 18 bit-flip
penalties -> weighted gather of 128 rows/token from a 262144 x 512 value
table -> output projection.

Sharding: data-parallel over tokens (512 tokens/core x 8 cores); the value
table is replicated into every core's HBM and rows are fetched with indirect
(gather) DMA.  The beam search runs on the Vector engine as a sorted-list
bitonic merge (pens kept ascending; pens+delta is then also ascending, so
each of the 18 rounds is one selection stage + 5 bitonic sort stages, with
the flip masks carried as fp32 payloads via predicated copies).  The
weighted sum runs on ScalarE (per-partition scale) + TensorE (identity
matmul accumulate into PSUM), split with VectorE where it overlaps best.
"""
import os
import sys

sys.path.insert(0, "/opt/trn_rl_repo")

import numpy as np

D = 1024
H = 4
NB = 18
BD = 16
KD = 288
KNN = 32
TK = 262144
V = 512
R = 512
JD = H * KD            # 1152, up-projection output dim
SD = H * NB * 2        # 144, score dim
NCORES = 8
N = 2 * 2048           # tokens
TPC = int(os.environ.get("BPCM_TPC", str(N // NCORES)))  # tokens per core
NG = TPC // 128        # token groups of 128
GH = max(NG // 2, 1)   # groups per beam half
NHALF = max(NG // GH, 1)
P = 128
CH = 16                # gather chunk: slots per indirect DMA
NCH = (H * KNN) // CH  # 8 chunks per token group

DEBUG = bool(int(os.environ.get("BPCM_DEBUG", "0")))
# dtype of the replicated value table ("f32" safe / "bf16" fast)
VALUES_DT = os.environ.get("BPCM_VALUES_DT", "f32")

_cache = {}


def _build(nc, mybir, bass, tile, ExitStack):
    from concourse.masks import make_identity

    dt = mybir.dt
    f32 = dt.float32
    i32 = dt.int32
    vdt = f32 if VALUES_DT == "f32" else dt.bfloat16
    Alu = mybir.AluOpType
    Act = mybir.ActivationFunctionType

    xT_d = nc.dram_tensor("xT", [D, TPC], f32, kind="ExternalInput").ap()
    wdT_d = nc.dram_tensor("WdT", [D, R], f32, kind="ExternalInput").ap()
    bdT_d = nc.dram_tensor("bdT", [P, R // P], f32, kind="ExternalInput").ap()
    wuT_d = nc.dram_tensor("WuT", [R, JD], f32, kind="ExternalInput").ap()
    wb_d = nc.dram_tensor("Wbig", [JD, SD], f32, kind="ExternalInput").ap()
    wpT_d = nc.dram_tensor("WpT", [V, D], f32, kind="ExternalInput").ap()
    val_d = nc.dram_tensor("values", [TK, V], vdt, kind="ExternalInput").ap()
    out_d = nc.dram_tensor("out", [TPC, D], f32, kind="ExternalOutput").ap()
    dbg = {}
    if DEBUG:
        for nm, shp in [("ls", [P, NG, SD]), ("delta", [P, NG, H, NB]),
                        ("bc", [P, NG, H]), ("pen", [P, NG, H, KNN]),
                        ("msk", [P, NG, H, KNN]), ("w", [P, NG, H, KNN]),
                        ("y", [P, NG, V])]:
            dbg[nm] = nc.dram_tensor("dbg_" + nm, shp, f32,
                                     kind="ExternalOutput").ap()
        dbg["idx"] = nc.dram_tensor("dbg_idx", [P, NG, H, KNN], i32,
                                    kind="ExternalOutput").ap()
        dbg["g00"] = nc.dram_tensor("dbg_g00", [P, V], f32,
                                    kind="ExternalOutput").ap()

    KC_D = D // P   # 8 contraction chunks for down proj
    MC_R = R // P   # 4 output chunks for down proj
    KC_R = R // P   # 4
    JC = JD // P    # 9
    VC = V // P     # 4

    with tile.TileContext(nc) as tc, ExitStack() as ctx:
        # ---------- constants + weight loads ----------
        cpool = ctx.enter_context(tc.tile_pool(name="const", bufs=1))
        ident = cpool.tile([P, P], f32)
        make_identity(nc, ident[:])
        identv = ident
        if vdt != f32:
            identv = cpool.tile([P, P], vdt)
            nc.vector.tensor_copy(identv[:], ident[:])
        bwt = cpool.tile([P, NB], f32)
        for m in range(NB):
            nc.vector.memset(bwt[:, m:m + 1], float(1 << m))

        # persistent small tiles (allocated before the big scoped pool)
        perpool = ctx.enter_context(tc.tile_pool(name="persist", bufs=1))
        wp_sb = perpool.tile([P, VC, D], f32)
        ls_sb = perpool.tile([P, NG, SD], f32)
        nc.sync.dma_start(wp_sb[:], wpT_d.rearrange("(k p) n -> p k n", p=P))

        # ---------- projections (PE, fp32) ----------
        with tc.tile_pool(name="projw", bufs=1) as wpool, \
                tc.tile_pool(name="projps", bufs=2, space="PSUM") as pps:
            xT_sb = wpool.tile([P, KC_D, TPC], f32)
            wd_sb = wpool.tile([P, KC_D, R], f32)
            bd_sb = wpool.tile([P, MC_R], f32)
            wu_sb = wpool.tile([P, KC_R, JD], f32)
            wb_sb = wpool.tile([P, JC, SD], f32)
            qd_sb = wpool.tile([P, MC_R, TPC], f32)
            q_sb = wpool.tile([P, JC, TPC], f32)
            nc.sync.dma_start(xT_sb[:], xT_d.rearrange("(k p) n -> p k n", p=P))
            nc.sync.dma_start(wd_sb[:], wdT_d.rearrange("(k p) n -> p k n", p=P))
            nc.sync.dma_start(bd_sb[:], bdT_d[:])
            nc.sync.dma_start(wu_sb[:], wuT_d.rearrange("(k p) n -> p k n", p=P))
            nc.sync.dma_start(wb_sb[:], wb_d.rearrange("(k p) n -> p k n", p=P))
            for mc in range(MC_R):
                ps = pps.tile([P, TPC], f32, tag="pp")
                for kc in range(KC_D):
                    nc.tensor.matmul(ps[:], wd_sb[:, kc, mc * P:(mc + 1) * P],
                                     xT_sb[:, kc, :],
                                     start=(kc == 0), stop=(kc == KC_D - 1))
                nc.scalar.activation(qd_sb[:, mc, :], ps[:], Act.Identity,
                                     bias=bd_sb[:, mc:mc + 1], scale=1.0)
            for jc in range(JC):
                ps = pps.tile([P, TPC], f32, tag="pp")
                for kc in range(KC_R):
                    nc.tensor.matmul(ps[:], wu_sb[:, kc, jc * P:(jc + 1) * P],
                                     qd_sb[:, kc, :],
                                     start=(kc == 0), stop=(kc == KC_R - 1))
                nc.scalar.copy(q_sb[:, jc, :], ps[:])
            for g in range(NG):
                ps2 = pps.tile([P, SD], f32, tag="pl")
                for jc in range(JC):
                    nc.tensor.matmul(ps2[:], q_sb[:, jc, g * P:(g + 1) * P],
                                     wb_sb[:, jc, :],
                                     start=(jc == 0), stop=(jc == JC - 1))
                nc.scalar.copy(ls_sb[:, g, :], ps2[:])

        # ---------- score post-processing (DVE) ----------
        lsv = ls_sb[:].rearrange("p g (h m c) -> p g h m c", h=H, c=2)
        spool = ctx.enter_context(tc.tile_pool(name="scorep", bufs=1))
        delta = spool.tile([P, NG, H, NB], f32)
        bb = spool.tile([P, NG, H, NB], f32)
        bc = spool.tile([P, NG, H], f32)
        nc.vector.tensor_tensor(out=delta[:], in0=lsv[:, :, :, :, 0],
                                in1=lsv[:, :, :, :, 1], op=Alu.subtract)
        nc.vector.scalar_tensor_tensor(out=delta[:], in0=delta[:], scalar=-1.0,
                                       in1=delta[:], op0=Alu.mult, op1=Alu.max)
        nc.vector.tensor_tensor(out=bb[:], in0=lsv[:, :, :, :, 1],
                                in1=lsv[:, :, :, :, 0], op=Alu.is_gt)
        bwb = bwt[:].unsqueeze(1).unsqueeze(1).to_broadcast([P, NG, H, NB])
        nc.vector.tensor_tensor(out=bb[:], in0=bb[:], in1=bwb, op=Alu.mult)
        nc.vector.tensor_reduce(out=bc[:], in_=bb[:], axis=mybir.AxisListType.X,
                                op=Alu.add)
        if DEBUG:
            nc.sync.dma_start(dbg["ls"][:], ls_sb[:])
            nc.sync.dma_start(dbg["delta"][:], delta[:])
            nc.sync.dma_start(dbg["bc"][:], bc[:])

        # global result tiles for the gather phase
        gpool = ctx.enter_context(tc.tile_pool(name="gres", bufs=1))
        w_sb = gpool.tile([P, NG, H, KNN], f32)
        idx_sb = gpool.tile([P, NG, H, KNN], i32)

        # ---------- beam search per half (DVE) ----------
        bpool = ctx.enter_context(tc.tile_pool(name="beam", bufs=2))

        def cmpx(pen, msk, cmp, tmp, W, d):
            """compare-exchange stage at distance d within sorted width W"""
            pv = pen[:, :, :, :W].rearrange("p g h (b two d) -> p g h b two d",
                                            two=2, d=d)
            mv = msk[:, :, :, :W].rearrange("p g h (b two d) -> p g h b two d",
                                            two=2, d=d)
            cv = cmp[:, :, :, :W // 2].rearrange("p g h (b d) -> p g h b d", d=d)
            tv = tmp[:, :, :, :W // 2].rearrange("p g h (b d) -> p g h b d", d=d)
            pA, pB = pv[:, :, :, :, 0, :], pv[:, :, :, :, 1, :]
            mA, mB = mv[:, :, :, :, 0, :], mv[:, :, :, :, 1, :]
            nc.vector.tensor_tensor(out=cv, in0=pA, in1=pB, op=Alu.is_gt)
            nc.vector.tensor_copy(tv, mA)
            nc.vector.copy_predicated(mA, cv, mB)
            nc.vector.copy_predicated(mB, cv, tv)
            nc.vector.tensor_copy(tv, pA)
            nc.vector.copy_predicated(pA, cv, pB)
            nc.vector.copy_predicated(pB, cv, tv)

        for hh in range(NHALF):
            g0 = hh * GH
            pen = bpool.tile([P, GH, H, KNN], f32, tag="pen")
            msk = bpool.tile([P, GH, H, KNN], f32, tag="msk")
            penB = bpool.tile([P, GH, H, KNN], f32, tag="penB")
            mskB = bpool.tile([P, GH, H, KNN], f32, tag="mskB")
            cmp = bpool.tile([P, GH, H, KNN], i32, tag="cmp")
            tmp = bpool.tile([P, GH, H, KNN], f32, tag="tmp")

            nc.vector.memset(pen[:, :, :, :1], 0.0)
            nc.vector.memset(msk[:, :, :, :1], 0.0)
            for t in range(NB):
                w = min(1 << t, KNN)
                dlt = delta[:, g0:g0 + GH, :, t:t + 1].to_broadcast(
                    [P, GH, H, w])
                nc.vector.tensor_tensor(out=penB[:, :, :, :w],
                                        in0=pen[:, :, :, :w], in1=dlt,
                                        op=Alu.add)
                nc.vector.tensor_scalar(out=mskB[:, :, :, :w],
                                        in0=msk[:, :, :, :w],
                                        scalar1=float(1 << t), scalar2=None,
                                        op0=Alu.add)
                if w < KNN:
                    # grow phase: [A | rev(B)] is bitonic of width 2w -> sort
                    nc.vector.tensor_copy(pen[:, :, :, w:2 * w],
                                          penB[:, :, :, w - 1::-1])
                    nc.vector.tensor_copy(msk[:, :, :, w:2 * w],
                                          mskB[:, :, :, w - 1::-1])
                    dstep = w
                    while dstep >= 1:
                        cmpx(pen, msk, cmp, tmp, 2 * w, dstep)
                        dstep //= 2
                else:
                    # selection: keep the 32 smallest of A u B elementwise
                    revB = penB[:, :, :, ::-1]
                    revmB = mskB[:, :, :, ::-1]
                    nc.vector.tensor_tensor(out=cmp[:], in0=pen[:], in1=revB,
                                            op=Alu.is_gt)
                    nc.vector.copy_predicated(pen[:], cmp[:], revB)
                    nc.vector.copy_predicated(msk[:], cmp[:], revmB)
                    if t < NB - 1:
                        dstep = KNN // 2
                        while dstep >= 1:
                            cmpx(pen, msk, cmp, tmp, KNN, dstep)
                            dstep //= 2

            # softmax(-pen) and idx = bc ^ msk
            eN = bpool.tile([P, GH, H, KNN], f32, tag="eN")
            sR = bpool.tile([P, GH, H], f32, tag="sR")
            mi = bpool.tile([P, GH, H, KNN], i32, tag="mi")
            bi = bpool.tile([P, GH, H], i32, tag="bi")
            nc.scalar.activation(eN[:], pen[:], Act.Exp, scale=-1.0)
            nc.vector.tensor_reduce(out=sR[:], in_=eN[:],
                                    axis=mybir.AxisListType.X, op=Alu.add)
            nc.vector.reciprocal(sR[:], sR[:])
            nc.vector.tensor_tensor(
                out=w_sb[:, g0:g0 + GH, :, :], in0=eN[:],
                in1=sR[:].unsqueeze(-1).to_broadcast([P, GH, H, KNN]),
                op=Alu.mult)
            nc.vector.tensor_copy(mi[:], msk[:])
            nc.vector.tensor_copy(bi[:], bc[:, g0:g0 + GH, :])
            nc.vector.tensor_tensor(
                out=idx_sb[:, g0:g0 + GH, :, :], in0=mi[:],
                in1=bi[:].unsqueeze(-1).to_broadcast([P, GH, H, KNN]),
                op=Alu.bitwise_xor)
            if DEBUG:
                nc.sync.dma_start(dbg["pen"][:, g0:g0 + GH], pen[:])
                nc.sync.dma_start(dbg["msk"][:, g0:g0 + GH], msk[:])

        if DEBUG:
            nc.sync.dma_start(dbg["w"][:], w_sb[:])
            nc.sync.dma_start(dbg["idx"][:], idx_sb[:])

        # ---------- gather + weighted sum + output projection ----------
        vpool = ctx.enter_context(tc.tile_pool(name="gath", bufs=16))
        ppool = ctx.enter_context(tc.tile_pool(name="scaled", bufs=6))
        ypool = ctx.enter_context(tc.tile_pool(name="yres", bufs=1))
        opool = ctx.enter_context(tc.tile_pool(name="outsb", bufs=2))
        wps = ctx.enter_context(tc.tile_pool(name="wsps", bufs=2, space="PSUM"))
        tps = ctx.enter_context(tc.tile_pool(name="trps", bufs=2, space="PSUM"))
        ops = ctx.enter_context(tc.tile_pool(name="oups", bufs=2, space="PSUM"))
        y_sb = ypool.tile([P, NG, V], f32)
        yT_sb = ypool.tile([P, VC, P], f32)

        # slot -> engine assignment: first half all PE, second half mixed
        def use_dve(g, s):
            return (g >= GH) and (s % 8 < 3)

        for g in range(NG):
            acc_ps = wps.tile([P, V], f32, tag="acc")
            acc_dv = (ypool.tile([P, V], f32, tag="accd", name="accd")
                      if g >= GH else None)
            pe_slots = [s for s in range(H * KNN) if not use_dve(g, s)]
            dv_slots = [s for s in range(H * KNN) if use_dve(g, s)]
            for s in range(H * KNN):
                gt = vpool.tile([P, V], vdt, tag="gt")
                h0, s0 = s // KNN, s % KNN
                nc.gpsimd.indirect_dma_start(
                    out=gt[:], out_offset=None, in_=val_d,
                    in_offset=bass.IndirectOffsetOnAxis(
                        ap=idx_sb[:, g, h0, s0:s0 + 1], axis=0))
                if DEBUG and g == 0 and s == 0 and vdt == f32:
                    nc.sync.dma_start(dbg["g00"][:], gt[:])
                w_ap = w_sb[:, g, h0, s0:s0 + 1]
                if s in dv_slots:
                    if s == dv_slots[0]:
                        nc.vector.tensor_scalar(
                            out=acc_dv[:], in0=gt[:], scalar1=w_ap,
                            scalar2=None, op0=Alu.mult)
                    else:
                        nc.vector.scalar_tensor_tensor(
                            out=acc_dv[:], in0=gt[:], scalar=w_ap,
                            in1=acc_dv[:], op0=Alu.mult, op1=Alu.add)
                else:
                    pt = ppool.tile([P, V], vdt, tag="pt")
                    nc.scalar.activation(pt[:], gt[:], Act.Identity,
                                         scale=w_ap)
                    nc.tensor.matmul(acc_ps[:],
                                     identv[:] if vdt != f32 else ident[:],
                                     pt[:],
                                     start=(s == pe_slots[0]),
                                     stop=(s == pe_slots[-1]))
            if dv_slots:
                nc.vector.tensor_tensor(out=y_sb[:, g, :], in0=acc_dv[:],
                                        in1=acc_ps[:], op=Alu.add)
            else:
                nc.scalar.copy(y_sb[:, g, :], acc_ps[:])
            if DEBUG:
                nc.sync.dma_start(dbg["y"][:, g], y_sb[:, g, :])

            # transpose y_g then project with WpT
            for vc in range(VC):
                pst = tps.tile([P, P], f32, tag="tr")
                nc.tensor.transpose(pst[:], y_sb[:, g, vc * P:(vc + 1) * P],
                                    ident[:])
                nc.scalar.copy(yT_sb[:, vc, :], pst[:])
            out_sb = opool.tile([P, D], f32, tag="ot")
            for dh in range(2):
                pso = ops.tile([P, D // 2], f32, tag="op")
                for vc in range(VC):
                    nc.tensor.matmul(pso[:], yT_sb[:, vc, :],
                                     wp_sb[:, vc, dh * 512:(dh + 1) * 512],
                                     start=(vc == 0), stop=(vc == VC - 1))
                nc.scalar.copy(out_sb[:, dh * 512:(dh + 1) * 512], pso[:])
            nc.sync.dma_start(out_d[g * P:(g + 1) * P, :], out_sb[:])


def _get_nc():
    key = (VALUES_DT, DEBUG)
    if key in _cache:
        return _cache[key]
    from contextlib import ExitStack
    from concourse import bacc, bass, mybir, tile
    nc = bacc.Bacc("TRN2", target_bir_lowering=False, debug=False,
                   num_devices=NCORES)
    _build(nc, mybir, bass, tile, ExitStack)
    nc.compile()
    _cache[key] = nc
    return nc


def _host_prep(x, keys, values, Wd, bd, Wu, Wp):
    xf = np.ascontiguousarray(np.asarray(x, np.float32).reshape(N, D))
    keys = np.asarray(keys, np.float32)
    W_big = np.zeros((JD, SD), np.float32)
    for h in range(H):
        for m in range(NB):
            for c in range(2):
                s = h * (NB * 2) + m * 2 + c
                W_big[h * KD + m * BD:h * KD + (m + 1) * BD, s] = keys[h, m, c]
    vdt = np.float32 if VALUES_DT == "f32" else None
    if vdt is np.float32:
        vals = np.ascontiguousarray(np.asarray(values, np.float32))
    else:
        import ml_dtypes
        vals = np.asarray(values, np.float32).astype(ml_dtypes.bfloat16)
    common = {
        "WdT": np.ascontiguousarray(np.asarray(Wd, np.float32).T),
        "bdT": np.ascontiguousarray(
            np.asarray(bd, np.float32).reshape(R // P, P).T),
        "WuT": np.ascontiguousarray(np.asarray(Wu, np.float32).T),
        "Wbig": W_big,
        "WpT": np.ascontiguousarray(np.asarray(Wp, np.float32).T),
        "values": vals,
    }
    in_maps = []
    for c in range(NCORES):
        m = dict(common)
        m["xT"] = np.ascontiguousarray(xf[c * TPC:(c + 1) * TPC].T)
        in_maps.append(m)
    return in_maps


def kernel(x, keys, values, Wd, bd, Wu, Wp):
    from concourse.bass_utils import run_bass_kernel_spmd
    nc = _get_nc()
    in_maps = _host_prep(x, keys, values, Wd, bd, Wu, Wp)
    res = run_bass_kernel_spmd(nc, in_maps, list(range(NCORES)))
    out = np.concatenate([res.results[c]["out"] for c in range(NCORES)],
                         axis=0)
    kernel.last_results = res
    return out.reshape(2, 2048, D).astype(np.float32)
